# revision 1
# baseline (speedup 1.0000x reference)
"""AffCoeffToMatrix TRN2 kernel (v5: pair-fused scalar chain, 3-engine balanced).

For each batch element (B = 2,000,000):
  R = rodrigues(rotat), U = rodrigues(scal_dir), D = exp(scal)
  M = R @ (U @ diag(D) @ U^T);  out = [M | trans]  -> [B, 3, 4] f32

Sharding: pure batch-parallel over 8 NeuronCores (no communication).
On-core layout: batch over [128 partitions x F free], T tiles in A/B pairs.
The transcendental chain runs on PAIR-WIDE [P,4F] tiles (one Ln, two Exp,
two Sin per pair) so the list scheduler cannot interleave Exp/Sin tables:
2 activation-table loads per pair, 6 per sweep. rth carries a ln(2) bias
(rth2 = 2/theta) so every coefficient folds into 2x tensor ops or 4x
tensor-scalar ops - no 1x scalar_tensor_tensor anywhere.
  ACT : squares, Ln/Exp, Sin pair, W^2, v deinterleave + t3 copies
  DVE : fp16 2x tensor ops (R|U paired rotation build, M=R@S row muls),
        4x two-scalar tensor_scalar coefficient ops
  Pool: theta^2 f32 adds, d-planes, W = U*e, S = W W^T assembly, final
        M-row add written straight into the interleaved f32 out tile.
"""
import math
import sys

for _p in ("/opt/trn_rl_repo", "/root/.axon_site/_ro/trn_rl_repo"):
    if _p not in sys.path:
        sys.path.append(_p)

import numpy as np

import concourse.bass as bass
import concourse.mybir as mybir
import concourse.tile as tile

F32 = mybir.dt.float32
F16 = mybir.dt.float16
AF = mybir.ActivationFunctionType
OP = mybir.AluOpType
PI = math.pi

# ---- hardcoded problem geometry ----
B = 2_000_000
N_CORES = 8
P = 128
F = 246            # free-dim elements per tile
T = 8              # tiles per core
L = F * T          # elements per partition lane
E = P * L          # elements per core
BPAD = N_CORES * E


def _split_multi_waits(nc, limit=1, drain_limit=0):
    """This container's walrus cannot encode >1 sync-wait per instruction
    (Drain: none at all). Spill extras onto same-engine NOPs."""
    for b in nc.main_func.blocks:
        new = []
        for ins in b.instructions:
            si = getattr(ins, "sync_info", None)
            waits = list(si.on_wait) if (si is not None and si.on_wait) else []
            lim = drain_limit if isinstance(ins, mybir.InstDrain) else limit
            if len(waits) > lim:
                keep, spill = waits[:lim], waits[lim:]
                for w in spill:
                    nop = mybir.InstNoOp(
                        name=nc.get_next_instruction_name(),
                        sync_info=mybir.SyncInfo(on_wait=[w], on_update=[]),
                        bass_nofuse=True,
                        engine=ins.engine,
                    )
                    nc.register_instruction(nop)
                    new.append(nop)
                ins.sync_info = mybir.SyncInfo(
                    on_wait=keep, on_update=list(si.on_update or [])
                )
            new.append(ins)
        b.instructions[:] = new


def build_module(F=F, T=T, loop_rep=None):
    nc = bass.Bass()
    E_ = P * F * T
    rot = nc.dram_tensor("rotat", [E_, 3], F32, kind="ExternalInput")
    sd = nc.dram_tensor("scal_dir", [E_, 3], F32, kind="ExternalInput")
    sc = nc.dram_tensor("scal", [E_, 3], F32, kind="ExternalInput")
    tr = nc.dram_tensor("trans", [E_, 3], F32, kind="ExternalInput")
    out = nc.dram_tensor("out", [E_, 12], F32, kind="ExternalOutput")

    L_ = F * T

    def make_views(w):
        t = L_ // w
        return {
            "rot": rot[:].rearrange("(t p f) c -> t p (f c)", t=t, p=P),
            "sd": sd[:].rearrange("(t p f) c -> t p (f c)", t=t, p=P),
            "sc": sc[:].rearrange("(t p f) c -> t p (f c)", t=t, p=P),
            "tr": tr[:].rearrange("(t p f) c -> t p (f c)", t=t, p=P),
            "out": out[:].rearrange("(t p f) c -> t p (f c)", t=t, p=P),
        }

    pairs = [(F, g) for g in range(0, T, 2)]
    V = {w: make_views(w) for w in {p[0] for p in pairs}}

    with tile.TileContext(nc) as tc:
        with (
            tc.tile_pool(name="pin", bufs=2) as pin,
            tc.tile_pool(name="pst", bufs=2) as pst,
            tc.tile_pool(name="p1", bufs=1) as p1,
            tc.tile_pool(name="pch", bufs=2) as pch,
            tc.tile_pool(name="pmat", bufs=1) as pmat,
            tc.tile_pool(name="pru", bufs=2) as pru,
            tc.tile_pool(name="ps9", bufs=2) as ps9,
            tc.tile_pool(name="pb1", bufs=1) as pb1,
            tc.tile_pool(name="pmm", bufs=1) as pmm,
            tc.tile_pool(name="pout", bufs=2) as pout,
            tc.tile_pool(name="pc", bufs=1) as pc,
        ):
            pi2 = pc.tile([P, 1], F32, tag="pi2")
            nc.vector.memset(pi2[:], PI / 2)
            ln2 = pc.tile([P, 1], F32, tag="ln2")
            nc.vector.memset(ln2[:], math.log(2.0))
            # dummy Ln warms the natural_log_exp table during the first DMA
            warm = pc.tile([P, 1], F32, tag="warm", name="warm")
            nc.scalar.activation(warm[:], pi2[:], AF.Ln)

            # pair state: scalar chain is [P, 4F] = (tileA 2F | tileB 2F),
            # each 2F half = (R rotation | U rotation) for that tile.
            def pre_pair(F, vw, g, first=False):
                F2 = 2 * F
                F4 = 4 * F
                s = {}
                rs = [None, None]
                v6 = [None, None]
                th2 = p1.tile([P, F4], F32, tag="th2", name="th2")
                t2v = th2[:].rearrange("p (q g f) -> p q g f", q=2, g=2)
                for h in (0, 1):
                    ti = g + h
                    rs6 = pin.tile([P, 6 * F], F32, tag=f"rs6{h}", name="rs6")
                    nc.sync.dma_start(out=rs6[:, : 3 * F], in_=vw["rot"][ti])
                    nc.sync.dma_start(out=rs6[:, 3 * F :], in_=vw["sd"][ti])
                    rs[h] = rs6
                    rsv = rs6[:].rearrange("p (g f c) -> p g c f", g=2, c=3)
                    # squares (ACT, table-free) -> planar f32
                    sq6 = p1.tile([P, 6 * F], F32, tag=f"sq6{h}", name="sq6")
                    sqv = sq6[:].rearrange("p (g c f) -> p g c f", g=2, c=3)
                    if first:
                        # pair 0: square the rot half while the sd DMA is
                        # still in flight
                        nc.scalar.activation(sqv[:, 0:1], rsv[:, 0:1], AF.Square)
                        nc.scalar.activation(sqv[:, 1:2], rsv[:, 1:2], AF.Square)
                    else:
                        nc.scalar.activation(sqv, rsv, AF.Square)
                    # theta^2 (Pool, f32) into the pair tile
                    th2a = p1.tile([P, F2], F32, tag=f"th2a{h}", name="th2a")
                    t2av = th2a[:].rearrange("p (g f) -> p g f", g=2)
                    if first:
                        # per-rotation-half adds start as soon as each
                        # half's squares land
                        nc.gpsimd.tensor_add(
                            t2av[:, 0:1], sqv[:, 0:1, 0, :], sqv[:, 0:1, 1, :]
                        )
                        nc.gpsimd.tensor_add(
                            t2v[:, h, 0:1], t2av[:, 0:1], sqv[:, 0:1, 2, :]
                        )
                        nc.gpsimd.tensor_add(
                            t2av[:, 1:2], sqv[:, 1:2, 0, :], sqv[:, 1:2, 1, :]
                        )
                        nc.gpsimd.tensor_add(
                            t2v[:, h, 1:2], t2av[:, 1:2], sqv[:, 1:2, 2, :]
                        )
                    else:
                        nc.gpsimd.tensor_add(
                            t2av, sqv[:, :, 0, :], sqv[:, :, 1, :]
                        )
                        nc.gpsimd.tensor_add(t2v[:, h], t2av, sqv[:, :, 2, :])
                sc6 = pb1.tile([P, 6 * F], F32, tag="sc6", name="sc6")
                nc.sync.dma_start(out=sc6[:, : 3 * F], in_=vw["sc"][g])
                nc.sync.dma_start(out=sc6[:, 3 * F :], in_=vw["sc"][g + 1])
                # ln/exp chain, pair-wide: th = sqrt(th2), rth2 = 2/sqrt(th2)
                lg = p1.tile([P, F4], F32, tag="lg", name="lg")
                nc.scalar.activation(lg[:], th2[:], AF.Ln)
                th = pst.tile([P, F4], F16, tag="th", name="th")
                nc.scalar.activation(th[:], lg[:], AF.Exp, scale=0.5)
                rth2 = pst.tile([P, F4], F16, tag="rth2", name="rth2")
                nc.scalar.activation(rth2[:], lg[:], AF.Exp, scale=-0.5, bias=ln2[:])
                e6 = pst.tile([P, 6 * F], F16, tag="e6", name="e6")
                e6v = e6[:].rearrange("p (q c f) -> p q c f", q=2, c=3)
                nc.scalar.activation(
                    e6v,
                    sc6[:].rearrange("p (q f c) -> p q c f", q=2, c=3),
                    AF.Exp,
                    scale=0.5,
                )
                s["e6"] = e6
                s["th"], s["rth2"], s["sc6"] = th, rth2, sc6
                s["rs"], s["v6"] = rs, v6
                return s

            def trig_pair(F, s, first=False):
                F2 = 2 * F
                F4 = 4 * F
                th = s["th"]
                sh = pch.tile([P, F4], F16, tag="sh", name="sh")
                nc.scalar.activation(sh[:], th[:], AF.Sin, scale=0.5)
                m4 = pch.tile([P, F4], F16, tag="m4", name="m4")
                nc.vector.tensor_scalar(m4[:], th[:], PI, -4 * PI, OP.is_gt, OP.mult)
                u4 = pch.tile([P, F4], F16, tag="u4", name="u4")
                nc.vector.tensor_add(u4[:], m4[:], th[:])
                ch = pch.tile([P, F4], F16, tag="ch", name="ch")
                nc.scalar.activation(ch[:], u4[:], AF.Sin, scale=0.5, bias=pi2[:])
                # coefficients, pair-wide (DVE 2x / 4x)
                t2 = pch.tile([P, F4], F16, tag="t2", name="t2")
                nc.vector.tensor_mul(t2[:], sh[:], s["rth2"][:])  # 2 sin(t/2)/t
                a2 = pch.tile([P, F4], F16, tag="a2", name="a2")
                nc.vector.tensor_mul(a2[:], t2[:], ch[:])         # sin(t)*2/t... = a
                tsq = pch.tile([P, F4], F16, tag="m4", name="tsq")
                nc.gpsimd.tensor_mul(tsq[:], t2[:], t2[:])
                b2 = pch.tile([P, F4], F16, tag="b2", name="b2")
                nc.vector.tensor_scalar(b2[:], tsq[:], 0.5, None, OP.mult)  # b
                shsq = pch.tile([P, F4], F16, tag="shsq", name="shsq")
                nc.vector.tensor_mul(shsq[:], sh[:], sh[:])
                c2 = pch.tile([P, F4], F16, tag="c2", name="c2")
                nc.vector.tensor_scalar(c2[:], shsq[:], -2.0, 1.0, OP.mult, OP.add)
                s["a2"], s["b2"], s["c2"] = a2, b2, c2
                # deinterleave v to planar fp16 (ACT copy, table-free),
                # emitted after the Sin pair so it never delays the
                # Ln->Exp->Sin critical chain
                for h in (0, 1):
                    rsv = s["rs"][h][:].rearrange("p (g f c) -> p g c f", g=2, c=3)
                    s["v6"][h] = pst.tile(
                        [P, 6 * F], F16, tag=f"v6{h}", name="v6"
                    )
                    v6v = s["v6"][h][:].rearrange("p (g c f) -> p g c f", g=2, c=3)
                    if first:
                        # pair 0: DVE idles waiting for the first Sin; give
                        # it the deinterleave (2x TensorCopy) to fill the gap
                        nc.vector.tensor_copy(v6v, rsv)
                    else:
                        nc.scalar.activation(v6v, rsv, AF.Copy)

            def mat(F, vw, ti, s, h, last=False):
                F2 = 2 * F
                v6 = s["v6"][h]
                v6v = v6[:].rearrange("p (g c f) -> p g c f", g=2, c=3)
                e3v = s["e6"][:, 3 * F * h : 3 * F * (h + 1)].rearrange(
                    "p (c f) -> p c f", c=3
                )
                a2 = s["a2"][:, F2 * h : F2 * (h + 1)]
                b2 = s["b2"][:, F2 * h : F2 * (h + 1)]
                c2 = s["c2"][:, F2 * h : F2 * (h + 1)]

                t3 = pin.tile([P, 3 * F], F32, tag="tr3", name="tr3")
                nc.sync.dma_start(out=t3[:], in_=vw["tr"][ti])
                ot = pout.tile([P, 12 * F], F32, tag="out", name="ot")
                otv = ot[:].rearrange("p (f c) -> p c f", c=12)
                nc.scalar.activation(
                    otv[:, 3:12:4, :],
                    t3[:].rearrange("p (f c) -> p c f", c=3),
                    AF.Copy,
                )

                def mt(tag, w):
                    return pmat.tile([P, w], F16, tag=tag, name=tag)

                def pair_b(ap_f2, n):
                    # [P, 2F] (R|U) -> [P, 2, n, F] broadcast over n
                    return (
                        ap_f2.rearrange("p (g f) -> p g f", g=2)
                        .unsqueeze(2)
                        .to_broadcast((P, 2, n, F))
                    )


                # rotation build, R|U paired fp16 (DVE 2x)
                bv6 = mt("bv6", 6 * F)
                bv6v = bv6[:].rearrange("p (g c f) -> p g c f", g=2, c=3)
                nc.vector.tensor_mul(bv6v, pair_b(b2, 3), v6v)
                # av planes in (z, x, y) order: avc[g,0]=a*z, avc[g,1:3]=a*(x,y)
                avc6 = mt("avc6", 6 * F)
                avc6v = avc6[:].rearrange("p (g c f) -> p g c f", g=2, c=3)
                nc.vector.tensor_mul(
                    avc6v[:, :, 0:1, :], pair_b(a2, 1), v6v[:, :, 2:3, :]
                )
                nc.gpsimd.tensor_mul(
                    avc6v[:, :, 1:3, :], pair_b(a2, 2), v6v[:, :, 0:2, :]
                )
                # d planes on Pool (balance)
                d6 = mt("d6", 6 * F)
                d6v = d6[:].rearrange("p (g c f) -> p g c f", g=2, c=3)
                nc.gpsimd.tensor_mul(d6v, bv6v, v6v)
                # p planes: (p01, p12, p20) = (bx*y, by*z, bz*x)
                p6 = mt("p6", 6 * F)
                p6v = p6[:].rearrange("p (g c f) -> p g c f", g=2, c=3)
                nc.vector.tensor_mul(
                    p6v[:, :, 0:2, :], bv6v[:, :, 0:2, :], v6v[:, :, 1:3, :]
                )
                nc.vector.tensor_mul(
                    p6v[:, :, 2:3, :], bv6v[:, :, 2:3, :], v6v[:, :, 0:1, :]
                )
                # RU18 = (R9 | U9), row-major
                RU18 = pru.tile([P, 18 * F], F16, tag="RU18", name="RU18")
                ruv = RU18[:].rearrange("p (g k f) -> p g k f", g=2, k=9)
                nc.vector.tensor_add(ruv[:, :, 0:9:4, :], d6v, pair_b(c2, 3))
                nc.vector.tensor_add(
                    ruv[:, :, 3:8:4, :], p6v[:, :, 0:2, :], avc6v[:, :, 0:2, :]
                )
                nc.vector.tensor_add(
                    ruv[:, :, 2, :], p6v[:, :, 2, :], avc6v[:, :, 2, :]
                )
                nc.vector.tensor_sub(
                    ruv[:, :, 1:6:4, :], p6v[:, :, 0:2, :], avc6v[:, :, 0:2, :]
                )
                nc.vector.tensor_sub(
                    ruv[:, :, 6, :], p6v[:, :, 2, :], avc6v[:, :, 2, :]
                )
                R9v = RU18[:, : 9 * F].rearrange("p (k f) -> p k f", k=9)
                U9v = RU18[:, 9 * F :].rearrange("p (i k f) -> p i k f", i=3, k=3)

                # scaling: W = U * diag(e) (Pool), squares (ACT)
                W9 = mt("W9", 9 * F)
                W9v4 = W9[:].rearrange("p (i k f) -> p i k f", i=3, k=3)
                e_b = e3v.unsqueeze(1).to_broadcast((P, 3, 3, F))
                nc.gpsimd.tensor_mul(W9v4, U9v, e_b)
                W9v = W9[:].rearrange("p (k f) -> p k f", k=9)
                sqW = mt("sqW", 9 * F)
                nc.scalar.activation(sqW[:], W9[:], AF.Square)
                sqWv = sqW[:].rearrange("p (i k f) -> p i k f", i=3, k=3)
                # S unique-6 layout: S00@0 S01@1 S02@2 S11@3 S12@5 S22@8
                S9 = ps9.tile([P, 9 * F], F16, tag="S9", name="S9")
                S9v = S9[:].rearrange("p (k f) -> p k f", k=9)
                sdt = mt("sdt", 3 * F)
                sdtv = sdt[:].rearrange("p (c f) -> p c f", c=3)
                nc.gpsimd.tensor_add(sdtv, sqWv[:, :, 0, :], sqWv[:, :, 1, :])
                nc.gpsimd.tensor_add(
                    S9v[:, 0:4:3, :], sdtv[:, 0:2, :], sqWv[:, 0:2, 2, :]
                )
                nc.gpsimd.tensor_add(S9v[:, 8, :], sdtv[:, 2, :], sqWv[:, 2, 2, :])
                pp = mt("pp", 9 * F)
                ppv = pp[:].rearrange("p (g k f) -> p g k f", g=3, k=3)
                nc.gpsimd.tensor_mul(ppv[:, 0, :, :], W9v[:, 0:3, :], W9v[:, 3:6, :])
                nc.gpsimd.tensor_mul(ppv[:, 1, :, :], W9v[:, 0:3, :], W9v[:, 6:9, :])
                nc.gpsimd.tensor_mul(ppv[:, 2, :, :], W9v[:, 3:6, :], W9v[:, 6:9, :])
                q3 = mt("q3", 3 * F)
                q3v = q3[:].rearrange("p (g f) -> p g f", g=3)
                nc.gpsimd.tensor_add(q3v, ppv[:, :, 0, :], ppv[:, :, 1, :])
                nc.gpsimd.tensor_add(
                    S9v[:, 1:3, :], q3v[:, 0:2, :], ppv[:, 0:2, 2, :]
                )
                nc.gpsimd.tensor_add(S9v[:, 5, :], q3v[:, 2, :], ppv[:, 2, 2, :])

                # M = R @ S: per-row muls (DVE), final add on Pool straight
                # into the interleaved f32 out tile
                srow = [S9v[:, 0:3, :], S9v[:, 1:7:2, :], S9v[:, 2:9:3, :]]
                # out rows live at (f c) positions c = 4i + j
                orows = ot[:].rearrange("p (f i c) -> p i c f", i=3, c=4)

                def bcast(ap_pf, n):
                    return ap_pf.unsqueeze(1).to_broadcast((P, n, ap_pf.shape[-1]))

                for i in range(3):
                    mp1 = mt("mp1", 3 * F)
                    mp1v = mp1[:].rearrange("p (c f) -> p c f", c=3)
                    nc.vector.tensor_mul(mp1v, bcast(R9v[:, 3 * i, :], 3), srow[0])
                    mp2 = mt("mp2", 3 * F)
                    mp2v = mp2[:].rearrange("p (c f) -> p c f", c=3)
                    nc.vector.tensor_mul(
                        mp2v, bcast(R9v[:, 3 * i + 1, :], 3), srow[1]
                    )
                    ms = pmm.tile([P, 3 * F], F16, tag="ms", name="ms")
                    msv = ms[:].rearrange("p (c f) -> p c f", c=3)
                    nc.vector.tensor_add(msv, mp1v, mp2v)
                    mp3 = pmm.tile([P, 3 * F], F16, tag="mp3", name="mp3")
                    mp3v = mp3[:].rearrange("p (c f) -> p c f", c=3)
                    nc.vector.tensor_mul(
                        mp3v, bcast(R9v[:, 3 * i + 2, :], 3), srow[2]
                    )
                    if not last:
                        nc.gpsimd.tensor_add(orows[:, i, 0:3, :], msv, mp3v)
                    else:
                        # drain the tail: finish the first f-half of all rows
                        # so its DMA overlaps the second half's final adds
                        Fh = F // 2
                        nc.gpsimd.tensor_add(
                            orows[:, i, 0:3, :Fh], msv[:, :, :Fh], mp3v[:, :, :Fh]
                        )
                        nc.gpsimd.tensor_add(
                            orows[:, i, 0:3, Fh:], msv[:, :, Fh:], mp3v[:, :, Fh:]
                        )
                if not last:
                    nc.sync.dma_start(out=vw["out"][ti], in_=ot[:])
                else:
                    Fh = F // 2
                    nc.sync.dma_start(
                        out=vw["out"][ti][:, : 6 * F], in_=ot[:, : 6 * F]
                    )
                    nc.sync.dma_start(
                        out=vw["out"][ti][:, 6 * F :], in_=ot[:, 6 * F :]
                    )

            def sweep():
                for (Fp, g) in pairs:
                    vw = V[Fp]
                    s = pre_pair(Fp, vw, g, first=(Fp, g) == pairs[0])
                    trig_pair(Fp, s, first=(Fp, g) == pairs[0])
                    mat(Fp, vw, g, s, 0)
                    mat(Fp, vw, g + 1, s, 1, last=(Fp, g) == pairs[-1])

            if loop_rep is None:
                sweep()
            else:
                with tc.For_i(0, loop_rep, 1, staggered_reset=True):
                    sweep()

    _split_multi_waits(nc)
    return nc


# ----------------------------------------------------------------------------
# host-side execution
# ----------------------------------------------------------------------------
_CACHE = {}


def _get_runner():
    if "runner" in _CACHE:
        return _CACHE["runner"]
    import jax
    from jax.sharding import Mesh, PartitionSpec
    from jax.experimental.shard_map import shard_map
    from concourse.bass2jax import (
        _bass_exec_p,
        install_neuronx_cc_hook,
        partition_id_tensor,
    )

    nc = build_module()
    install_neuronx_cc_hook()
    partition_name = nc.partition_id_tensor.name if nc.partition_id_tensor else None
    in_names, out_names, out_avals, zero_outs = [], [], [], []
    for alloc in nc.m.functions[0].allocations:
        if not isinstance(alloc, mybir.MemoryLocationSet):
            continue
        name = alloc.memorylocations[0].name
        if alloc.kind == "ExternalInput":
            if name != partition_name:
                in_names.append(name)
        elif alloc.kind == "ExternalOutput":
            shape = tuple(alloc.tensor_shape)
            dtype = mybir.dt.np(alloc.dtype)
            out_names.append(name)
            out_avals.append(jax.core.ShapedArray(shape, dtype))
            zero_outs.append(np.zeros(shape, dtype))
    n_params = len(in_names)
    all_in_names = in_names + out_names + (
        [partition_name] if partition_name else []
    )

    def _body(*args):
        operands = list(args)
        if partition_name is not None:
            operands.append(partition_id_tensor())
        outs = _bass_exec_p.bind(
            *operands,
            out_avals=tuple(out_avals),
            in_names=tuple(all_in_names),
            out_names=tuple(out_names),
            lowering_input_output_aliases=(),
            sim_require_finite=True,
            sim_require_nnan=True,
            nc=nc,
        )
        return tuple(outs)

    devices = jax.devices()[:N_CORES]
    mesh = Mesh(np.asarray(devices), ("core",))
    n_outs = len(out_names)
    jf = jax.jit(
        shard_map(
            _body,
            mesh=mesh,
            in_specs=(PartitionSpec("core"),) * (n_params + n_outs),
            out_specs=(PartitionSpec("core"),) * n_outs,
            check_rep=False,
        ),
        donate_argnums=tuple(range(n_params, n_params + n_outs)),
        keep_unused=True,
    )
    _CACHE["runner"] = (jf, in_names, out_names, zero_outs)
    return _CACHE["runner"]


def kernel(trans, rotat, scal_dir, scal):
    jf, in_names, out_names, zero_outs = _get_runner()
    inputs = {"trans": trans, "rotat": rotat, "scal_dir": scal_dir, "scal": scal}
    # pad to BPAD with ones (zeros would make |v| = 0 -> inf/NaN chains)
    padded = {}
    for k, v in inputs.items():
        a = np.ones((BPAD, 3), dtype=np.float32)
        a[:B] = v
        padded[k] = a
    args = [padded[n] for n in in_names]
    zeros = [np.zeros((N_CORES * z.shape[0], *z.shape[1:]), z.dtype) for z in zero_outs]
    outs = jf(*args, *zeros)
    full = np.asarray(outs[0])  # [BPAD, 12]
    return full[:B].reshape(B, 3, 4).astype(np.float32, copy=False)


if __name__ == "__main__":
    rng = np.random.default_rng(0)
    ins = {
        "trans": rng.normal(size=(B, 3)).astype(np.float32),
        "rotat": rng.normal(size=(B, 3)).astype(np.float32),
        "scal_dir": rng.normal(size=(B, 3)).astype(np.float32),
        "scal": rng.normal(size=(B, 3)).astype(np.float32),
    }
    out = kernel(**ins)
    print(out.shape, out.dtype)

